# revision 25
# baseline (speedup 1.0000x reference)
"""Trainium2 Bass kernel for the AttentionBlock problem.

Reference semantics (shapes hardcoded):
    x [4, 256, 64, 64]; 1x1-conv weights q_w/k_w/v_w [256, 258] (+biases),
    fc_w [256, 256], fc_b [256].
    x0 = concat(x, pos) -> [B, 258, 4096]
    q/k/v = relu(W @ x0 + b)                    [B, 256, 4096]
    attn  = softmax_causal(q^T k)               [B, 4096, 4096]
    out   = x + relu(fc_w @ (attn @ v^T)^T + fc_b)

Distribution: 8 cores = 4 batches x 2 query-block roles. Each core
computes full k / v^T for its batch, q only for its 4 owned 512-wide
query blocks, and causal attention for those blocks. Causal work is
balanced by giving role 0 global blocks [0,3,4,7] and role 1 blocks
[1,2,5,6]; both roles run the identical SPMD program with per-slot
key-tile counts [8,16,24,32] (slightly padded), with per-core mask
data zeroing padded/non-causal entries.

Softmax is computed without max-subtraction (scores are ~26+-5, far
from fp32 overflow): p = exp(s) * mask, normalized by a replicated
ones-matmul denominator.

Precision split: the score path (q/k projections, q^T k) runs in
float32r; v/av/den/fc run in bf16 (error enters linearly there).
CRITICAL HW quirk (measured): f32r matmuls do not register as PE
activity for the HAM clock gate, so an all-f32r phase runs at the
cold 1.2GHz clock forever (v2 of this kernel did: phase A went
53us -> 115us). The bf16 v-projection matmuls interleaved between
f32r k-projection groups are what keep the PE at 2.4GHz, so v must
stay bf16 and adjacent to k in program order.

DMA design: all inputs are host-packed so each logical load is one
large dma_start (a dma_start costs ~730ns of Sync-sequencer issue
time; v1 had ~105 = 76us of issue, v3 has ~25). The bf16 copy of x
needed by the v-projection is derived on-device from the f32 copy by
the (otherwise idle in phase A) DVE instead of being DMA'd; the
128-partition zero-padded positional tiles that the f32r pos matmuls
need (4-partition f32r matmuls are ~3x slower - measured 865ns) are
built on-device with a gpsimd memset of rows 4:128 plus a tiny
4-row DMA. Per-core input drops from ~16MB (v1) to ~9MB, removing
the 13us phase-A PE starvation the v1 trace showed.
"""

import numpy as np

B = 4
C = 256
S = 64
N = S * S            # 4096
K = 256              # q/k/v channels
NBLK = 512           # query block width
NSLOT = 4            # owned query blocks per core
M_S = (8, 16, 24, 32)  # key-tile count per slot (128-wide key tiles)
BLOCKS = ((0, 3, 4, 7), (1, 2, 5, 6))  # role -> global block ids

_PROGRAM = None


def _build_program():
    import concourse.bacc as bacc
    import concourse.mybir as mybir
    import concourse.tile as tile

    F32 = mybir.dt.float32
    F32R = mybir.dt.float32r
    BF16 = mybir.dt.bfloat16
    FP16 = mybir.dt.float16
    Act = mybir.ActivationFunctionType

    nc = bacc.Bacc("TRN2", target_bir_lowering=False, debug=False)

    # x packed: col = pair*2048 + half*1024 + (n % 1024)
    xf_d = nc.dram_tensor("xf", [128, 8192], FP16, kind="ExternalInput")
    # owned-block x: col = slot*1024 + half*512 + (n_rel % 512)
    xq_d = nc.dram_tensor("xq", [128, 4096], FP16, kind="ExternalInput")
    # pos rows [px; py; 1; 0] for owned blocks: col = slot*512 + n_rel
    xqp_d = nc.dram_tensor("xqp", [4, 2048], FP16, kind="ExternalInput")
    # global pos rows [px; py; 1; 0]
    posf_d = nc.dram_tensor("posf", [4, 4096], FP16, kind="ExternalInput")
    # [wq_h0|wq_h1|wk_h0|wk_h1], each [128, 256]
    wall_d = nc.dram_tensor("wall", [128, 1024], FP16, kind="ExternalInput")
    # pos-weight rows [w_px; w_py; bias; 0] x [q|k], each [4, 256]
    wpos_d = nc.dram_tensor("wpos", [4, 512], FP16, kind="ExternalInput")
    # bf16: [pos rows (4096) | wv-pos (256)]
    posb_d = nc.dram_tensor("posb", [4, 4352], BF16, kind="ExternalInput")
    # bf16: [fcwT_v0 (256) | fcwT_v1 (256) | ones (128) | wv_h0 | wv_h1]
    wb_d = nc.dram_tensor("wb", [128, 1152], BF16, kind="ExternalInput")
    fcb_d = nc.dram_tensor("fcb", [128, 2], F32, kind="ExternalInput")
    # masks packed: col = slot*4096 + t*512 + q_rel  (t = last-8 tile idx)
    msk_d = nc.dram_tensor("masks", [128, 16384], BF16, kind="ExternalInput")
    out_d = nc.dram_tensor("out", [C, NSLOT * NBLK], F32, kind="ExternalOutput")

    with tile.TileContext(nc) as tc:
        with (
            tc.tile_pool(name="wts", bufs=1) as wts,
            tc.tile_pool(name="x0_p", bufs=2) as x0_p,
            tc.tile_pool(name="kqv_p", bufs=1) as kqv_p,
            tc.tile_pool(name="msk_p", bufs=2) as msk_p,
            tc.tile_pool(name="ex_p", bufs=9) as ex_p,
            tc.tile_pool(name="ds_p", bufs=2) as ds_p,
            tc.tile_pool(name="o_p", bufs=4) as o_p,
            tc.tile_pool(name="rb_p", bufs=2) as rb_p,
            tc.tile_pool(name="tr_p", bufs=2) as tr_p,
            tc.tile_pool(name="ps_sc", bufs=4, space="PSUM") as ps_sc,
            tc.tile_pool(name="ps_out", bufs=1, space="PSUM") as ps_out,
            tc.tile_pool(name="ps_den", bufs=1, space="PSUM") as ps_den,
            tc.tile_pool(name="ps_fc", bufs=1, space="PSUM") as ps_fc,
        ):
            def wtile(dram, dt, tag):
                t = wts.tile(list(dram.shape), dt, tag=tag, name=tag)
                nc.sync.dma_start(t[:], dram[:, :])
                return t

            def padzero(cols, tag, dt):
                # [128, cols] tile, zeroed; rows 0:4 DMA'd over it later.
                # (engine partition ranges must be 32-aligned, so memset the
                # whole tile and let the 4-row DMA overwrite rows 0:4;
                # sub-128-partition matmul operands are NOT an option: the
                # 32-row tile mode measured +30us whole-kernel)
                t = wts.tile([128, cols], dt, tag=tag, name=tag)
                nc.gpsimd.memzero(t[:])
                return t

            def padfill(t, dram, dcols=None):
                nc.sync.dma_start(t[0:4, :], dram[:, :] if dcols is None
                                  else dram[:, dcols[0]:dcols[1]])

            # memzeros first (gpsimd, starts at preamble end), smallest
            # first so the sync queue's fill-DMA dependency waits are short
            wposp_t = padzero(512, "wposp", FP16)
            posp_t = padzero(4096, "posp", FP16)
            # bf16 pos rows / v-pos weights, 128-padded (4-partition bf16
            # matmuls measured 210ns vs 155ns for 128-partition at N=256)
            posbp_t = padzero(4096, "posbp", BF16)
            wvposp_t = padzero(256, "wvposp", BF16)
            xqpp_t = padzero(2048, "xqpp", FP16)

            # weights needed by phase A first so PE can start early
            wall_t = wtile(wall_d, FP16, "wall")

            # wall column offsets
            WQ, WK = 0, 512
            PQ, PK = 0, 256
            FCW, ONES, WV = 0, 512, 640

            # ---- phase A: k and vT per position-block pair ----
            k_sb = [[None] * 8 for _ in range(2)]
            vT_sb = [None] * 32

            def load_pair(p):
                xt = x0_p.tile([128, 2048], FP16, tag="xf", name=f"xf_{p}")
                # two half DMAs: the first k matmul only needs the h0 half,
                # so it can start 0.5MB of DMA earlier
                nc.sync.dma_start(xt[:, 0:1024],
                                  xf_d[:, 2048 * p:2048 * p + 1024])
                nc.sync.dma_start(xt[:, 1024:2048],
                                  xf_d[:, 2048 * p + 1024:2048 * (p + 1)])
                return xt

            def emit_pair(p, xt=None):
                if xt is None:
                    xt = load_pair(p)
                # bf16 copy for the v-projection's stationary operands
                xb = x0_p.tile([128, 2048], BF16, tag="xb", name=f"xb_{p}")
                nc.vector.tensor_copy(xb[:], xt[:])

                for li in range(2):
                    nb = 2 * p + li
                    for kt in range(2):
                        pk = ps_sc.tile([128, NBLK], F32, tag="sc",
                                        name=f"pk{kt}_{nb}")
                        nc.tensor.matmul(
                            pk[:], wall_t[:, WK + 128 * kt:WK + 128 * (kt + 1)],
                            xt[:, 512 * li:512 * (li + 1)],
                            start=True, stop=False)
                        nc.tensor.matmul(
                            pk[:],
                            wall_t[:, WK + 256 + 128 * kt:WK + 256 + 128 * (kt + 1)],
                            xt[:, 1024 + 512 * li:1024 + 512 * (li + 1)],
                            start=False, stop=False)
                        nc.tensor.matmul(
                            pk[:], wposp_t[:, PK + 128 * kt:PK + 128 * (kt + 1)],
                            posp_t[:, 512 * nb:512 * (nb + 1)],
                            start=False, stop=True)
                        kt_sb = kqv_p.tile([128, NBLK], FP16,
                                           tag=f"k{kt}_{nb}",
                                           name=f"k{kt}_{nb}")
                        nc.scalar.activation(kt_sb[:], pk[:], Act.Relu)
                        k_sb[kt][nb] = kt_sb
                for li in range(2):
                    nb = 2 * p + li
                    for sub in range(4):
                        i = 4 * nb + sub
                        pv = ps_sc.tile([128, K], F32, tag="sc",
                                        name=f"pv{i}")
                        nc.tensor.matmul(
                            pv[:],
                            xb[:, 512 * li + 128 * sub:512 * li + 128 * (sub + 1)],
                            wb_t[:, WV:WV + 256], start=True, stop=False)
                        nc.tensor.matmul(
                            pv[:],
                            xb[:, 1024 + 512 * li + 128 * sub:
                               1024 + 512 * li + 128 * (sub + 1)],
                            wb_t[:, WV + 256:WV + 512],
                            start=False, stop=False)
                        nc.tensor.matmul(
                            pv[:],
                            posbp_t[:, 512 * nb + 128 * sub:
                                    512 * nb + 128 * (sub + 1)],
                            wvposp_t[:], start=False, stop=True)
                        vt_sb = kqv_p.tile([128, K], BF16, tag=f"v{i}",
                                           name=f"v{i}")
                        nc.scalar.activation(vt_sb[:], pv[:], Act.Relu)
                        vT_sb[i] = vt_sb

            # pair-0 x DMAs first (they gate the first k matmul); wb
            # (carrying wv) only gates the v matmuls that follow ~5us later
            xt0 = load_pair(0)
            # pad fills AFTER the pair-0 x descriptors are in the queues;
            # their memzero waits block the sync queue, so everything from
            # here on is delayed until ~the posp memzero completes, which
            # is fine (wb/xq/pair-1 are needed much later)
            padfill(wposp_t, wpos_d)
            padfill(posp_t, posf_d)
            padfill(posbp_t, posb_d, dcols=(0, 4096))
            padfill(wvposp_t, posb_d, dcols=(4096, 4352))
            # split wb: the v-projection only needs the wv columns now;
            # fcw/ones are phase-B and can load behind pair-1
            wb_t = wts.tile([128, 1152], BF16, tag="wb", name="wb")
            nc.sync.dma_start(wb_t[:, 640:1152], wb_d[:, 640:1152])
            emit_pair(0, xt0)

            # secondary inputs, ordered by first use
            emit_pair(1)
            nc.sync.dma_start(wb_t[:, 0:640], wb_d[:, 0:640])
            emit_pair(2)
            xq_t = wtile(xq_d, FP16, "xq")
            emit_pair(3)
            padfill(xqpp_t, xqp_d)
            fcb_t = wtile(fcb_d, F32, "fcb")

            msk_t = [None] * NSLOT

            def emit_mask(s):
                mt = msk_p.tile([128, 4096], BF16, tag="mk", name=f"mk{s}")
                nc.sync.dma_start(mt[:], msk_d[:, 4096 * s:4096 * (s + 1)])
                msk_t[s] = mt

            emit_mask(0)

            # ---- phase A part 2: q per slot ----
            q_sb = [[None] * NSLOT for _ in range(2)]
            for s in range(NSLOT):
                for kt in range(2):
                    pq = ps_sc.tile([128, NBLK], F32, tag="sc",
                                    name=f"pq{kt}_{s}")
                    nc.tensor.matmul(
                        pq[:], wall_t[:, WQ + 128 * kt:WQ + 128 * (kt + 1)],
                        xq_t[:, 1024 * s:1024 * s + 512],
                        start=True, stop=False)
                    nc.tensor.matmul(
                        pq[:],
                        wall_t[:, WQ + 256 + 128 * kt:WQ + 256 + 128 * (kt + 1)],
                        xq_t[:, 1024 * s + 512:1024 * s + 1024],
                        start=False, stop=False)
                    nc.tensor.matmul(
                        pq[:], wposp_t[:, PQ + 128 * kt:PQ + 128 * (kt + 1)],
                        xqpp_t[:, 512 * s:512 * (s + 1)],
                        start=False, stop=True)
                    qt = kqv_p.tile([128, NBLK], FP16, tag=f"q{kt}_{s}",
                                    name=f"q{kt}_{s}")
                    nc.scalar.activation(qt[:], pq[:], Act.Relu)
                    q_sb[kt][s] = qt

            # ---- phase B: attention + fc per slot ----
            def finalize_slot(s, po, pd, nh=1):
                """fc on the unnormalized po, then scale/relu/residual/dma.

                fc(po*rb) == fc(po)*rb (column scaling commutes with the
                left matmul), so the fc matmuls depend only on a plain
                po->sbuf copy, not on the recip/normalize chain - the PE
                never waits on the DVE at slot finalize. nh=2 splits the
                post-fc chain into column halves (shorter tail on the very
                last slot).
                """
                o_sb = []
                for vt in range(2):
                    ot = o_p.tile([128, NBLK], BF16, tag=f"o{vt}",
                                  name=f"o{vt}_{s}")
                    nc.scalar.copy(ot[:], po[vt][:])
                    o_sb.append(ot)
                rb_sb = rb_p.tile([128, NBLK], F32, tag="rb", name=f"rb{s}")
                nc.vector.reciprocal_approx_fast(rb_sb[:], pd[:])
                pfc_t = []
                for ot in range(2):
                    pfc = ps_fc.tile([128, NBLK], F32, tag="fc",
                                     name=f"pfc{ot}_{s}")
                    for vt in range(2):
                        nc.tensor.matmul(
                            pfc[:],
                            wb_t[:, FCW + 256 * vt + 128 * ot:
                                 FCW + 256 * vt + 128 * (ot + 1)],
                            o_sb[vt][:], start=(vt == 0), stop=(vt == 1))
                    pfc_t.append(pfc)
                W = NBLK // nh
                for h in range(nh):
                    for ot in range(2):
                        hs = slice(W * h, W * (h + 1))
                        u_sb = tr_p.tile([128, W], F32, tag=f"u{ot}{h}",
                                         name=f"u{ot}_{s}_{h}")
                        nc.vector.tensor_mul(u_sb[:], pfc_t[ot][:, hs],
                                             rb_sb[:, hs])
                        t_sb = tr_p.tile([128, W], F32, tag=f"t{ot}{h}",
                                         name=f"t{ot}_{s}_{h}")
                        nc.scalar.activation(t_sb[:], u_sb[:], Act.Relu,
                                             bias=fcb_t[:, ot:ot + 1])
                        r_sb = tr_p.tile([128, W], F32, tag=f"r{ot}{h}",
                                         name=f"r{ot}_{s}_{h}")
                        nc.vector.tensor_add(
                            r_sb[:], t_sb[:],
                            xq_t[:, 1024 * s + 512 * ot + W * h:
                                 1024 * s + 512 * ot + W * (h + 1)])
                        nc.sync.dma_start(
                            out_d[128 * ot:128 * (ot + 1),
                                  NBLK * s + W * h:NBLK * s + W * (h + 1)],
                            r_sb[:])

            pending = None  # deferred finalize of previous slot
            for s in range(NSLOT):
                M = M_S[s]
                po = [ps_out.tile([128, NBLK], F32, tag=f"o{vt}",
                                  name=f"po{vt}_{s}") for vt in range(2)]
                pd = ps_den.tile([128, NBLK], F32, tag="den", name=f"pd{s}")
                ex_tiles = [None] * M

                def emit_scores(i, s=s, ex_tiles=ex_tiles, M=M):
                    # scores^T tile [128 keys, 512 queries]
                    psc = ps_sc.tile([128, NBLK], F32, tag="sc",
                                     name=f"psc{s}_{i}")
                    for kt in range(2):
                        nc.tensor.matmul(
                            psc[:],
                            k_sb[kt][i // 4][:, 128 * (i % 4):128 * (i % 4 + 1)],
                            q_sb[kt][s][:], start=(kt == 0), stop=(kt == 1))
                    ex = ex_p.tile([128, NBLK], BF16, tag="ex",
                                   name=f"ex{s}_{i}")
                    nc.scalar.activation(ex[:], psc[:], Act.Exp)
                    if i >= M - 8:
                        t = i - (M - 8)
                        nc.vector.tensor_mul(
                            ex[:], ex[:],
                            msk_t[s][:, 512 * t:512 * (t + 1)])
                    ex_tiles[i] = ex

                dshalf = [None]

                def consume_quad(j, po=po, pd=pd, M=M, ex_tiles=ex_tiles,
                                 s=s, dshalf=dshalf):
                    # octet-summed denominator: one den matmul per 8 tiles
                    # (the den matmul rides inside the existing bf16 batch).
                    # In the last quad the den matmul goes first so the
                    # finalize recip/mul overlap the trailing av matmuls.
                    da = ds_p.tile([128, NBLK], BF16, tag="ds",
                                   name=f"da{s}_{j}")
                    nc.vector.tensor_add(da[:], ex_tiles[j][:],
                                         ex_tiles[j + 1][:])
                    db = ds_p.tile([128, NBLK], BF16, tag="ds",
                                   name=f"db{s}_{j}")
                    nc.vector.tensor_add(db[:], ex_tiles[j + 2][:],
                                         ex_tiles[j + 3][:])
                    # alternating tag: the stashed even-quad dsum must not
                    # share a ring slot with the odd-quad dsum that joins it
                    dsum = ds_p.tile([128, NBLK], BF16,
                                     tag=f"ds{(j // 4) % 2}",
                                     name=f"ds{s}_{j}")
                    nc.vector.tensor_add(dsum[:], da[:], db[:])

                    def den():
                        if j % 8 == 0:  # first quad of octet: stash
                            dshalf[0] = dsum
                            return
                        doct = ds_p.tile([128, NBLK], BF16, tag="do",
                                         name=f"do{s}_{j}")
                        nc.vector.tensor_add(doct[:], dshalf[0][:], dsum[:])
                        nc.tensor.matmul(pd[:], wb_t[:, ONES:ONES + 128],
                                         doct[:],
                                         start=(j == 4), stop=(j == M - 4))

                    if j == M - 4:
                        den()
                    for jj in range(j, j + 4):
                        e = ex_tiles[jj]
                        for vt in range(2):
                            nc.tensor.matmul(
                                po[vt][:],
                                vT_sb[jj][:, 128 * vt:128 * (vt + 1)],
                                e[:], start=(jj == 0), stop=(jj == M - 1))
                    if j != M - 4:
                        den()
                    for jj in range(j, j + 4):
                        ex_tiles[jj] = None

                # 4-tile score batches between bf16 consume batches: fewer
                # f32r<->bf16 PE dtype switches (each costs ~100-200ns)
                for ib in range(0, M, 4):
                    if ib == 0 and s < NSLOT - 1:
                        emit_mask(s + 1)
                    for i in range(ib, ib + 4):
                        emit_scores(i)
                    if ib == 4 and pending is not None:
                        finalize_slot(*pending)
                        pending = None
                    if ib >= 4:
                        consume_quad(ib - 4)
                consume_quad(M - 4)
                pending = (s, po, pd)

            finalize_slot(*pending, nh=2)

    nc.compile()
    return nc


def _host_prep(x, q_w, q_b, k_w, k_b, v_w, v_b, fc_w, fc_b):
    """Build the per-core input maps."""
    import ml_dtypes
    f32 = np.float32
    bf16 = ml_dtypes.bfloat16
    n = np.arange(N)
    px = ((n // S) / S).astype(f32)
    py = ((n % S) / S).astype(f32)
    pos4 = np.stack([px, py, np.ones(N, f32), np.zeros(N, f32)])  # [4, N]

    def pack_w(w):
        # [256 chan, 256 out] -> [wh0 | wh1] as [128, 512]
        wT = np.ascontiguousarray(w.astype(f32).T[:C])
        return np.concatenate([wT[:128], wT[128:]], axis=1)

    def pos_w(w, b):
        return np.stack([w.astype(f32).T[C], w.astype(f32).T[C + 1],
                         b.astype(f32), np.zeros(K, f32)])

    wall = np.concatenate([pack_w(q_w), pack_w(k_w)], axis=1)
    wpos = np.concatenate([pos_w(q_w, q_b), pos_w(k_w, k_b)], axis=1)
    posb = np.concatenate([pos4, pos_w(v_w, v_b)], axis=1).astype(bf16)
    fcwT = np.ascontiguousarray(fc_w.astype(f32).T)
    wb = np.concatenate([fcwT[:128], fcwT[128:], np.ones((128, 128), f32),
                         pack_w(v_w)], axis=1).astype(bf16)
    fcb = np.ascontiguousarray(fc_b.astype(f32).reshape(2, 128).T)

    # per-role masks, packed [128, NSLOT*8*512]
    mm = np.arange(128)[:, None]
    nn = np.arange(NBLK)[None, :]
    masks = {}
    for r in range(2):
        mr = np.zeros((NSLOT, 8, 128, NBLK), f32)
        for s in range(NSLOT):
            j = BLOCKS[r][s]
            for t in range(8):
                i = M_S[s] - 8 + t
                mr[s, t] = (128 * i + mm <= 512 * j + nn)
        masks[r] = np.ascontiguousarray(
            mr.transpose(2, 0, 1, 3).reshape(128, NSLOT * 8 * NBLK)
        ).astype(bf16)

    f16 = np.float16
    shared = {
        "wall": np.ascontiguousarray(wall).astype(f16),
        "wpos": np.ascontiguousarray(wpos).astype(f16),
        "posb": posb,
        "wb": wb,
        "fcb": fcb,
        "posf": pos4.astype(f16),
    }

    in_maps = []
    for c in range(8):
        b, r = c // 2, c % 2
        xb = x[b].reshape(C, N).astype(f32)
        # xf: col = pair*2048 + half*1024 + (n % 1024)
        xf = np.ascontiguousarray(
            xb.reshape(2, 128, 4, 1024).transpose(1, 2, 0, 3).reshape(128, 8192))
        xq_cols = np.concatenate(
            [np.arange(NBLK * j, NBLK * (j + 1)) for j in BLOCKS[r]])
        xg = xb[:, xq_cols]
        xq = np.ascontiguousarray(
            xg.reshape(2, 128, 4, 512).transpose(1, 2, 0, 3).reshape(128, 4096))
        xqp = np.ascontiguousarray(pos4[:, xq_cols])
        in_maps.append(dict(
            shared, xf=xf.astype(f16), xq=xq.astype(f16),
            xqp=xqp.astype(f16), masks=masks[r],
        ))
    return in_maps


def _gather(results):
    out = np.empty((B, C, N), np.float32)
    for c in range(8):
        b, r = c // 2, c % 2
        oc = results[c]["out"]
        for s, j in enumerate(BLOCKS[r]):
            out[b][:, NBLK * j:NBLK * (j + 1)] = oc[:, NBLK * s:NBLK * (s + 1)]
    return out.reshape(B, C, S, S)


def run(trace=False, **inputs):
    from concourse import bass_utils
    global _PROGRAM
    if _PROGRAM is None:
        _PROGRAM = _build_program()
    in_maps = _host_prep(**inputs)
    res = bass_utils.run_bass_kernel_spmd(
        _PROGRAM, in_maps, list(range(8)), trace=trace)
    return _gather(res.results), res


def kernel(**inputs):
    out, _ = run(trace=False, **inputs)
    return out


# revision 26
# speedup vs baseline: 1.0055x; 1.0055x over previous
"""Trainium2 Bass kernel for the AttentionBlock problem.

Reference semantics (shapes hardcoded):
    x [4, 256, 64, 64]; 1x1-conv weights q_w/k_w/v_w [256, 258] (+biases),
    fc_w [256, 256], fc_b [256].
    x0 = concat(x, pos) -> [B, 258, 4096]
    q/k/v = relu(W @ x0 + b)                    [B, 256, 4096]
    attn  = softmax_causal(q^T k)               [B, 4096, 4096]
    out   = x + relu(fc_w @ (attn @ v^T)^T + fc_b)

Distribution: 8 cores = 4 batches x 2 query-block roles. Each core
computes full k / v^T for its batch, q only for its 4 owned 512-wide
query blocks, and causal attention for those blocks. Causal work is
balanced by giving role 0 global blocks [0,3,4,7] and role 1 blocks
[1,2,5,6]; both roles run the identical SPMD program with per-slot
key-tile counts [8,16,24,32] (slightly padded), with per-core mask
data zeroing padded/non-causal entries.

Softmax is computed without max-subtraction (scores are ~26+-5, far
from fp32 overflow): p = exp(s) * mask, normalized by a replicated
ones-matmul denominator.

Precision split: the score path (q/k projections, q^T k) runs in
float32r; v/av/den/fc run in bf16 (error enters linearly there).
CRITICAL HW quirk (measured): f32r matmuls do not register as PE
activity for the HAM clock gate, so an all-f32r phase runs at the
cold 1.2GHz clock forever (v2 of this kernel did: phase A went
53us -> 115us). The bf16 v-projection matmuls interleaved between
f32r k-projection groups are what keep the PE at 2.4GHz, so v must
stay bf16 and adjacent to k in program order.

DMA design: all inputs are host-packed so each logical load is one
large dma_start (a dma_start costs ~730ns of Sync-sequencer issue
time; v1 had ~105 = 76us of issue, v3 has ~25). The bf16 copy of x
needed by the v-projection is derived on-device from the f32 copy by
the (otherwise idle in phase A) DVE instead of being DMA'd; the
128-partition zero-padded positional tiles that the f32r pos matmuls
need (4-partition f32r matmuls are ~3x slower - measured 865ns) are
built on-device with a gpsimd memset of rows 4:128 plus a tiny
4-row DMA. Per-core input drops from ~16MB (v1) to ~9MB, removing
the 13us phase-A PE starvation the v1 trace showed.
"""

import numpy as np

B = 4
C = 256
S = 64
N = S * S            # 4096
K = 256              # q/k/v channels
NBLK = 512           # query block width
NSLOT = 4            # owned query blocks per core
M_S = (8, 16, 24, 32)  # key-tile count per slot (128-wide key tiles)
BLOCKS = ((0, 3, 4, 7), (1, 2, 5, 6))  # role -> global block ids

_PROGRAM = None


def _build_program():
    import concourse.bacc as bacc
    import concourse.mybir as mybir
    import concourse.tile as tile

    F32 = mybir.dt.float32
    F32R = mybir.dt.float32r
    BF16 = mybir.dt.bfloat16
    FP16 = mybir.dt.float16
    Act = mybir.ActivationFunctionType

    nc = bacc.Bacc("TRN2", target_bir_lowering=False, debug=False)

    # x packed: col = pair*2048 + half*1024 + (n % 1024)
    xf_d = nc.dram_tensor("xf", [128, 8192], FP16, kind="ExternalInput")
    # owned-block x: col = slot*1024 + half*512 + (n_rel % 512)
    xq_d = nc.dram_tensor("xq", [128, 4096], FP16, kind="ExternalInput")
    # pos rows [px; py; 1; 0] for owned blocks: col = slot*512 + n_rel
    xqp_d = nc.dram_tensor("xqp", [4, 2048], FP16, kind="ExternalInput")
    # global pos rows [px; py; 1; 0]
    posf_d = nc.dram_tensor("posf", [4, 4096], FP16, kind="ExternalInput")
    # [wq_h0|wq_h1|wk_h0|wk_h1], each [128, 256]
    wall_d = nc.dram_tensor("wall", [128, 1024], FP16, kind="ExternalInput")
    # pos-weight rows [w_px; w_py; bias; 0] x [q|k], each [4, 256]
    wpos_d = nc.dram_tensor("wpos", [4, 512], FP16, kind="ExternalInput")
    # bf16: [pos rows (4096) | wv-pos (256)]
    posb_d = nc.dram_tensor("posb", [4, 4352], BF16, kind="ExternalInput")
    # bf16: [fcwT_v0 (256) | fcwT_v1 (256) | ones (128) | wv_h0 | wv_h1]
    wb_d = nc.dram_tensor("wb", [128, 1152], BF16, kind="ExternalInput")
    fcb_d = nc.dram_tensor("fcb", [128, 2], F32, kind="ExternalInput")
    # masks packed: col = slot*4096 + t*512 + q_rel  (t = last-8 tile idx)
    msk_d = nc.dram_tensor("masks", [128, 16384], BF16, kind="ExternalInput")
    out_d = nc.dram_tensor("out", [C, NSLOT * NBLK], F32, kind="ExternalOutput")

    with tile.TileContext(nc) as tc:
        with (
            tc.tile_pool(name="wts", bufs=1) as wts,
            tc.tile_pool(name="x0_p", bufs=2) as x0_p,
            tc.tile_pool(name="kqv_p", bufs=1) as kqv_p,
            tc.tile_pool(name="msk_p", bufs=2) as msk_p,
            tc.tile_pool(name="ex_p", bufs=9) as ex_p,
            tc.tile_pool(name="ds_p", bufs=2) as ds_p,
            tc.tile_pool(name="o_p", bufs=4) as o_p,
            tc.tile_pool(name="rb_p", bufs=2) as rb_p,
            tc.tile_pool(name="tr_p", bufs=2) as tr_p,
            tc.tile_pool(name="ps_sc", bufs=4, space="PSUM") as ps_sc,
            tc.tile_pool(name="ps_out", bufs=1, space="PSUM") as ps_out,
            tc.tile_pool(name="ps_den", bufs=1, space="PSUM") as ps_den,
            tc.tile_pool(name="ps_fc", bufs=1, space="PSUM") as ps_fc,
        ):
            def wtile(dram, dt, tag):
                t = wts.tile(list(dram.shape), dt, tag=tag, name=tag)
                nc.sync.dma_start(t[:], dram[:, :])
                return t

            def padzero(cols, tag, dt):
                # [128, cols] tile, zeroed; rows 0:4 DMA'd over it later.
                # (engine partition ranges must be 32-aligned, so memset the
                # whole tile and let the 4-row DMA overwrite rows 0:4;
                # sub-128-partition matmul operands are NOT an option: the
                # 32-row tile mode measured +30us whole-kernel)
                t = wts.tile([128, cols], dt, tag=tag, name=tag)
                nc.gpsimd.memzero(t[:])
                return t

            def padfill(t, dram, dcols=None):
                nc.sync.dma_start(t[0:4, :], dram[:, :] if dcols is None
                                  else dram[:, dcols[0]:dcols[1]])

            # memzeros first (gpsimd, starts at preamble end), smallest
            # first so the sync queue's fill-DMA dependency waits are short
            wposp_t = padzero(512, "wposp", FP16)
            posp_t = padzero(4096, "posp", FP16)
            # bf16 pos rows / v-pos weights, 128-padded (4-partition bf16
            # matmuls measured 210ns vs 155ns for 128-partition at N=256)
            posbp_t = padzero(4096, "posbp", BF16)
            wvposp_t = padzero(256, "wvposp", BF16)
            xqpp_t = padzero(2048, "xqpp", FP16)

            # weights needed by phase A first so PE can start early
            wall_t = wtile(wall_d, FP16, "wall")

            # wall column offsets
            WQ, WK = 0, 512
            PQ, PK = 0, 256
            FCW, ONES, WV = 0, 512, 640

            # ---- phase A: k and vT per position-block pair ----
            k_sb = [[None] * 8 for _ in range(2)]
            vT_sb = [None] * 32

            def load_pair(p):
                xt = x0_p.tile([128, 2048], FP16, tag="xf", name=f"xf_{p}")
                # two half DMAs: the first k matmul only needs the h0 half,
                # so it can start 0.5MB of DMA earlier
                nc.sync.dma_start(xt[:, 0:1024],
                                  xf_d[:, 2048 * p:2048 * p + 1024])
                nc.sync.dma_start(xt[:, 1024:2048],
                                  xf_d[:, 2048 * p + 1024:2048 * (p + 1)])
                return xt

            def emit_pair(p, xt=None):
                if xt is None:
                    xt = load_pair(p)
                # bf16 copy for the v-projection's stationary operands
                xb = x0_p.tile([128, 2048], BF16, tag="xb", name=f"xb_{p}")
                nc.vector.tensor_copy(xb[:], xt[:])

                for li in range(2):
                    nb = 2 * p + li
                    for kt in range(2):
                        pk = ps_sc.tile([128, NBLK], F32, tag="sc",
                                        name=f"pk{kt}_{nb}")
                        nc.tensor.matmul(
                            pk[:], wall_t[:, WK + 128 * kt:WK + 128 * (kt + 1)],
                            xt[:, 512 * li:512 * (li + 1)],
                            start=True, stop=False)
                        nc.tensor.matmul(
                            pk[:],
                            wall_t[:, WK + 256 + 128 * kt:WK + 256 + 128 * (kt + 1)],
                            xt[:, 1024 + 512 * li:1024 + 512 * (li + 1)],
                            start=False, stop=False)
                        nc.tensor.matmul(
                            pk[:], wposp_t[:, PK + 128 * kt:PK + 128 * (kt + 1)],
                            posp_t[:, 512 * nb:512 * (nb + 1)],
                            start=False, stop=True)
                        kt_sb = kqv_p.tile([128, NBLK], FP16,
                                           tag=f"k{kt}_{nb}",
                                           name=f"k{kt}_{nb}")
                        nc.scalar.activation(kt_sb[:], pk[:], Act.Relu)
                        k_sb[kt][nb] = kt_sb
                for li in range(2):
                    nb = 2 * p + li
                    for sub in range(4):
                        i = 4 * nb + sub
                        pv = ps_sc.tile([128, K], F32, tag="sc",
                                        name=f"pv{i}")
                        nc.tensor.matmul(
                            pv[:],
                            xb[:, 512 * li + 128 * sub:512 * li + 128 * (sub + 1)],
                            wb_t[:, WV:WV + 256], start=True, stop=False)
                        nc.tensor.matmul(
                            pv[:],
                            xb[:, 1024 + 512 * li + 128 * sub:
                               1024 + 512 * li + 128 * (sub + 1)],
                            wb_t[:, WV + 256:WV + 512],
                            start=False, stop=False)
                        nc.tensor.matmul(
                            pv[:],
                            posbp_t[:, 512 * nb + 128 * sub:
                                    512 * nb + 128 * (sub + 1)],
                            wvposp_t[:], start=False, stop=True)
                        vt_sb = kqv_p.tile([128, K], BF16, tag=f"v{i}",
                                           name=f"v{i}")
                        nc.scalar.activation(vt_sb[:], pv[:], Act.Relu)
                        vT_sb[i] = vt_sb

            # pair-0 x DMAs first (they gate the first k matmul); wb
            # (carrying wv) only gates the v matmuls that follow ~5us later
            xt0 = load_pair(0)
            # pad fills AFTER the pair-0 x descriptors are in the queues;
            # their memzero waits block the sync queue, so everything from
            # here on is delayed until ~the posp memzero completes, which
            # is fine (wb/xq/pair-1 are needed much later)
            padfill(wposp_t, wpos_d)
            padfill(posp_t, posf_d)
            padfill(posbp_t, posb_d, dcols=(0, 4096))
            padfill(wvposp_t, posb_d, dcols=(4096, 4352))
            # split wb: the v-projection only needs the wv columns now;
            # fcw/ones are phase-B and can load behind pair-1
            wb_t = wts.tile([128, 1152], BF16, tag="wb", name="wb")
            nc.sync.dma_start(wb_t[:, 640:1152], wb_d[:, 640:1152])
            emit_pair(0, xt0)

            # secondary inputs, ordered by first use
            emit_pair(1)
            nc.sync.dma_start(wb_t[:, 0:640], wb_d[:, 0:640])
            emit_pair(2)
            xq_t = wtile(xq_d, FP16, "xq")
            emit_pair(3)
            padfill(xqpp_t, xqp_d)
            fcb_t = wtile(fcb_d, F32, "fcb")

            msk_t = [None] * NSLOT

            def emit_mask(s):
                mt = msk_p.tile([128, 4096], BF16, tag="mk", name=f"mk{s}")
                nc.sync.dma_start(mt[:], msk_d[:, 4096 * s:4096 * (s + 1)])
                msk_t[s] = mt

            emit_mask(0)

            # ---- phase A part 2: q per slot ----
            q_sb = [[None] * NSLOT for _ in range(2)]
            for s in range(NSLOT):
                for kt in range(2):
                    pq = ps_sc.tile([128, NBLK], F32, tag="sc",
                                    name=f"pq{kt}_{s}")
                    nc.tensor.matmul(
                        pq[:], wall_t[:, WQ + 128 * kt:WQ + 128 * (kt + 1)],
                        xq_t[:, 1024 * s:1024 * s + 512],
                        start=True, stop=False)
                    nc.tensor.matmul(
                        pq[:],
                        wall_t[:, WQ + 256 + 128 * kt:WQ + 256 + 128 * (kt + 1)],
                        xq_t[:, 1024 * s + 512:1024 * s + 1024],
                        start=False, stop=False)
                    nc.tensor.matmul(
                        pq[:], wposp_t[:, PQ + 128 * kt:PQ + 128 * (kt + 1)],
                        xqpp_t[:, 512 * s:512 * (s + 1)],
                        start=False, stop=True)
                    qt = kqv_p.tile([128, NBLK], FP16, tag=f"q{kt}_{s}",
                                    name=f"q{kt}_{s}")
                    nc.scalar.activation(qt[:], pq[:], Act.Relu)
                    q_sb[kt][s] = qt

            # ---- phase B: attention + fc per slot ----
            def finalize_slot(s, po, pd, nh=1):
                """fc on the unnormalized po, then scale/relu/residual/dma.

                fc(po*rb) == fc(po)*rb (column scaling commutes with the
                left matmul), so the fc matmuls depend only on a plain
                po->sbuf copy, not on the recip/normalize chain - the PE
                never waits on the DVE at slot finalize. nh=2 splits the
                post-fc chain into column halves (shorter tail on the very
                last slot).
                """
                o_sb = []
                for vt in range(2):
                    ot = o_p.tile([128, NBLK], BF16, tag=f"o{vt}",
                                  name=f"o{vt}_{s}")
                    nc.scalar.copy(ot[:], po[vt][:])
                    o_sb.append(ot)
                rb_sb = rb_p.tile([128, NBLK], F32, tag="rb", name=f"rb{s}")
                nc.vector.reciprocal_approx_fast(rb_sb[:], pd[:])
                pfc_t = []
                for ot in range(2):
                    pfc = ps_fc.tile([128, NBLK], F32, tag="fc",
                                     name=f"pfc{ot}_{s}")
                    for vt in range(2):
                        nc.tensor.matmul(
                            pfc[:],
                            wb_t[:, FCW + 256 * vt + 128 * ot:
                                 FCW + 256 * vt + 128 * (ot + 1)],
                            o_sb[vt][:], start=(vt == 0), stop=(vt == 1))
                    pfc_t.append(pfc)
                W = NBLK // nh
                for h in range(nh):
                    for ot in range(2):
                        hs = slice(W * h, W * (h + 1))
                        u_sb = tr_p.tile([128, W], F32, tag=f"u{ot}{h}",
                                         name=f"u{ot}_{s}_{h}")
                        nc.vector.tensor_mul(u_sb[:], pfc_t[ot][:, hs],
                                             rb_sb[:, hs])
                        t_sb = tr_p.tile([128, W], F32, tag=f"t{ot}{h}",
                                         name=f"t{ot}_{s}_{h}")
                        nc.scalar.activation(t_sb[:], u_sb[:], Act.Relu,
                                             bias=fcb_t[:, ot:ot + 1])
                        r_sb = tr_p.tile([128, W], F32, tag=f"r{ot}{h}",
                                         name=f"r{ot}_{s}_{h}")
                        nc.vector.tensor_add(
                            r_sb[:], t_sb[:],
                            xq_t[:, 1024 * s + 512 * ot + W * h:
                                 1024 * s + 512 * ot + W * (h + 1)])
                        # alternate DGE queues: each dma_start costs
                        # ~600-1500ns of issue time, serial per queue
                        eng = nc.sync if (h + ot) % 2 == 0 else nc.scalar
                        eng.dma_start(
                            out_d[128 * ot:128 * (ot + 1),
                                  NBLK * s + W * h:NBLK * s + W * (h + 1)],
                            r_sb[:])

            pending = None  # deferred finalize of previous slot
            for s in range(NSLOT):
                M = M_S[s]
                po = [ps_out.tile([128, NBLK], F32, tag=f"o{vt}",
                                  name=f"po{vt}_{s}") for vt in range(2)]
                pd = ps_den.tile([128, NBLK], F32, tag="den", name=f"pd{s}")
                ex_tiles = [None] * M

                def emit_scores(i, s=s, ex_tiles=ex_tiles, M=M):
                    # scores^T tile [128 keys, 512 queries]
                    psc = ps_sc.tile([128, NBLK], F32, tag="sc",
                                     name=f"psc{s}_{i}")
                    for kt in range(2):
                        nc.tensor.matmul(
                            psc[:],
                            k_sb[kt][i // 4][:, 128 * (i % 4):128 * (i % 4 + 1)],
                            q_sb[kt][s][:], start=(kt == 0), stop=(kt == 1))
                    ex = ex_p.tile([128, NBLK], BF16, tag="ex",
                                   name=f"ex{s}_{i}")
                    nc.scalar.activation(ex[:], psc[:], Act.Exp)
                    if i >= M - 8:
                        t = i - (M - 8)
                        nc.vector.tensor_mul(
                            ex[:], ex[:],
                            msk_t[s][:, 512 * t:512 * (t + 1)])
                    ex_tiles[i] = ex

                dshalf = [None]

                def consume_quad(j, po=po, pd=pd, M=M, ex_tiles=ex_tiles,
                                 s=s, dshalf=dshalf):
                    # octet-summed denominator: one den matmul per 8 tiles
                    # (the den matmul rides inside the existing bf16 batch).
                    # In the last quad the den matmul goes first so the
                    # finalize recip/mul overlap the trailing av matmuls.
                    da = ds_p.tile([128, NBLK], BF16, tag="ds",
                                   name=f"da{s}_{j}")
                    nc.vector.tensor_add(da[:], ex_tiles[j][:],
                                         ex_tiles[j + 1][:])
                    db = ds_p.tile([128, NBLK], BF16, tag="ds",
                                   name=f"db{s}_{j}")
                    nc.vector.tensor_add(db[:], ex_tiles[j + 2][:],
                                         ex_tiles[j + 3][:])
                    # alternating tag: the stashed even-quad dsum must not
                    # share a ring slot with the odd-quad dsum that joins it
                    dsum = ds_p.tile([128, NBLK], BF16,
                                     tag=f"ds{(j // 4) % 2}",
                                     name=f"ds{s}_{j}")
                    nc.vector.tensor_add(dsum[:], da[:], db[:])

                    def den():
                        if j % 8 == 0:  # first quad of octet: stash
                            dshalf[0] = dsum
                            return
                        doct = ds_p.tile([128, NBLK], BF16, tag="do",
                                         name=f"do{s}_{j}")
                        nc.vector.tensor_add(doct[:], dshalf[0][:], dsum[:])
                        nc.tensor.matmul(pd[:], wb_t[:, ONES:ONES + 128],
                                         doct[:],
                                         start=(j == 4), stop=(j == M - 4))

                    if j == M - 4:
                        den()
                    for jj in range(j, j + 4):
                        e = ex_tiles[jj]
                        for vt in range(2):
                            nc.tensor.matmul(
                                po[vt][:],
                                vT_sb[jj][:, 128 * vt:128 * (vt + 1)],
                                e[:], start=(jj == 0), stop=(jj == M - 1))
                    if j != M - 4:
                        den()
                    for jj in range(j, j + 4):
                        ex_tiles[jj] = None

                # 4-tile score batches between bf16 consume batches: fewer
                # f32r<->bf16 PE dtype switches (each costs ~100-200ns)
                for ib in range(0, M, 4):
                    if ib == 0 and s < NSLOT - 1:
                        emit_mask(s + 1)
                    for i in range(ib, ib + 4):
                        emit_scores(i)
                    if ib == 4 and pending is not None:
                        finalize_slot(*pending)
                        pending = None
                    if ib >= 4:
                        consume_quad(ib - 4)
                consume_quad(M - 4)
                pending = (s, po, pd)

            finalize_slot(*pending, nh=2)

    nc.compile()
    return nc


def _host_prep(x, q_w, q_b, k_w, k_b, v_w, v_b, fc_w, fc_b):
    """Build the per-core input maps."""
    import ml_dtypes
    f32 = np.float32
    bf16 = ml_dtypes.bfloat16
    n = np.arange(N)
    px = ((n // S) / S).astype(f32)
    py = ((n % S) / S).astype(f32)
    pos4 = np.stack([px, py, np.ones(N, f32), np.zeros(N, f32)])  # [4, N]

    def pack_w(w):
        # [256 chan, 256 out] -> [wh0 | wh1] as [128, 512]
        wT = np.ascontiguousarray(w.astype(f32).T[:C])
        return np.concatenate([wT[:128], wT[128:]], axis=1)

    def pos_w(w, b):
        return np.stack([w.astype(f32).T[C], w.astype(f32).T[C + 1],
                         b.astype(f32), np.zeros(K, f32)])

    wall = np.concatenate([pack_w(q_w), pack_w(k_w)], axis=1)
    wpos = np.concatenate([pos_w(q_w, q_b), pos_w(k_w, k_b)], axis=1)
    posb = np.concatenate([pos4, pos_w(v_w, v_b)], axis=1).astype(bf16)
    fcwT = np.ascontiguousarray(fc_w.astype(f32).T)
    wb = np.concatenate([fcwT[:128], fcwT[128:], np.ones((128, 128), f32),
                         pack_w(v_w)], axis=1).astype(bf16)
    fcb = np.ascontiguousarray(fc_b.astype(f32).reshape(2, 128).T)

    # per-role masks, packed [128, NSLOT*8*512]
    mm = np.arange(128)[:, None]
    nn = np.arange(NBLK)[None, :]
    masks = {}
    for r in range(2):
        mr = np.zeros((NSLOT, 8, 128, NBLK), f32)
        for s in range(NSLOT):
            j = BLOCKS[r][s]
            for t in range(8):
                i = M_S[s] - 8 + t
                mr[s, t] = (128 * i + mm <= 512 * j + nn)
        masks[r] = np.ascontiguousarray(
            mr.transpose(2, 0, 1, 3).reshape(128, NSLOT * 8 * NBLK)
        ).astype(bf16)

    f16 = np.float16
    shared = {
        "wall": np.ascontiguousarray(wall).astype(f16),
        "wpos": np.ascontiguousarray(wpos).astype(f16),
        "posb": posb,
        "wb": wb,
        "fcb": fcb,
        "posf": pos4.astype(f16),
    }

    in_maps = []
    for c in range(8):
        b, r = c // 2, c % 2
        xb = x[b].reshape(C, N).astype(f32)
        # xf: col = pair*2048 + half*1024 + (n % 1024)
        xf = np.ascontiguousarray(
            xb.reshape(2, 128, 4, 1024).transpose(1, 2, 0, 3).reshape(128, 8192))
        xq_cols = np.concatenate(
            [np.arange(NBLK * j, NBLK * (j + 1)) for j in BLOCKS[r]])
        xg = xb[:, xq_cols]
        xq = np.ascontiguousarray(
            xg.reshape(2, 128, 4, 512).transpose(1, 2, 0, 3).reshape(128, 4096))
        xqp = np.ascontiguousarray(pos4[:, xq_cols])
        in_maps.append(dict(
            shared, xf=xf.astype(f16), xq=xq.astype(f16),
            xqp=xqp.astype(f16), masks=masks[r],
        ))
    return in_maps


def _gather(results):
    out = np.empty((B, C, N), np.float32)
    for c in range(8):
        b, r = c // 2, c % 2
        oc = results[c]["out"]
        for s, j in enumerate(BLOCKS[r]):
            out[b][:, NBLK * j:NBLK * (j + 1)] = oc[:, NBLK * s:NBLK * (s + 1)]
    return out.reshape(B, C, S, S)


def run(trace=False, **inputs):
    from concourse import bass_utils
    global _PROGRAM
    if _PROGRAM is None:
        _PROGRAM = _build_program()
    in_maps = _host_prep(**inputs)
    res = bass_utils.run_bass_kernel_spmd(
        _PROGRAM, in_maps, list(range(8)), trace=trace)
    return _gather(res.results), res


def kernel(**inputs):
    out, _ = run(trace=False, **inputs)
    return out
